# revision 13
# baseline (speedup 1.0000x reference)
"""Conditional Instance Norm (CIN) kernel for Trainium2, data-parallel over batch.

Reference semantics (per batch sample b, channel c):
    gamma_mix = style_weights @ gammas          # [B, C]
    beta_mix  = style_weights @ betas           # [B, C]
    y[b,c]    = gamma_mix[b,c] * (x[b,c] - mean) * rsqrt(var + eps) + beta_mix[b,c]
with mean/var over the spatial dims of x[b,c] (biased var).

Strategy: one batch sample per NeuronCore (B=8 samples, 8 cores).  HBM I/O is
fp16 (host converts): rel-err budget is 2e-2 and fp16 quantization costs
~1e-3, so this halves the memory-roofline floor vs fp32 — 32 MiB read +
32 MiB write per core instead of 64+64.  Channels are processed in tiles of
G channels; each channel's HW elements are laid out over Q=128/G partitions,
so a tile is a dense [128, F=HW/Q] fp16 SBUF block read from HBM exactly once
and written exactly once.

Per tile (default stats="act_square", act_frac=0.5, xt_bufs=10):
  DVE reduce_sum               -> per-partition sums   [128,1] f32
  ACT Square w/ accum_out      -> per-partition sumsq  [128,1] f32 (square
                                  result dumped to a fp16 scratch, never read)
  PE matmul w/ 1/HW selector   -> per-channel (mean, E[x^2])  [G,2]
  tiny DVE/ACT ops             -> scale = gamma*rsqrt(var+eps),
                                  bias  = beta - mean*scale   [G,2]
  PE matmul w/ 0/1 expander    -> per-partition (scale, bias) [128,2]
  apply y = scale*x + bias, split 50/50 between ACT (Identity w/ scale+bias
  APs) and DVE (tensor_scalar mult+add, fast fp16 mode) so neither engine
  exceeds the per-tile HBM budget; each segment is stored as soon as done.

PE Matmult instructions only tolerate a single sync-wait, so every matmul
operand is funneled through a DVE-produced tile (one wait, one engine).

HW notes (measured on trn2, slope bench):
  f32 baseline 404us; fp16 DMA-only copy ~210-216us (fp16 DMA runs at
  bytes-rate); this kernel ~228us.  tensor_tensor_reduce CRASHES on HW
  (works in CoreSim) — do not use.  tensor_scalar with accum_out works on
  HW but its mandatory full-size output write makes it a net loss vs
  reduce_sum (ts_square/dve_ts schemes measured 277us).  Pool (gpsimd)
  tensor_reduce only supports partition-axis (C) reductions.  TimelineSim
  under-predicts HW: f32 +8%, fp16 engine-heavy configs +20-50%.
"""

import sys

for _p in ("/opt/trn_rl_repo",):
    if _p not in sys.path:
        sys.path.insert(0, _p)

from contextlib import ExitStack

import numpy as np

import concourse.bacc as bacc
import concourse.tile as tile
from concourse import mybir
from concourse.bass_utils import run_bass_kernel_spmd

EPS = 1e-5

# Full problem dims (hardcoded per harness contract).
B, C, H, W = 8, 256, 256, 256
S = 16
HW = H * W
N_CORES = 8
P = 128  # SBUF partitions

AF = mybir.ActivationFunctionType
f32 = mybir.dt.float32
f16 = mybir.dt.float16


def _const_layout(C_, S_, G):
    """Column offsets of the packed constants tensor: g4 | e4 | gammas | betas | sw."""
    o_g4 = 0
    o_e4 = o_g4 + G
    o_gam = o_e4 + P
    o_bet = o_gam + C_
    o_sw = o_bet + C_
    ncols = o_sw + 1
    return o_g4, o_e4, o_gam, o_bet, o_sw, ncols


# act_frac: fraction of the apply pass done on the Scalar engine (rest on DVE).
# stats: "act_square" = DVE reduce_sum + ACT Square/accum (two full passes);
#        "dve_ts"     = DVE tensor_scalar*1.0/accum (sum) + DVE ttr x*x/accum
#                       (sumsq) — both candidates for the 2x/4x fp16 DVE modes,
#                       leaving ACT only the apply.
DEFAULTS = dict(G=16, xt_bufs=10, act_frac=0.5, stats="act_square")


def build_cin_program(
    C_=C,
    HW_=HW,
    S_=S,
    G=DEFAULTS["G"],  # channels per tile
    xt_bufs=DEFAULTS["xt_bufs"],
    act_frac=DEFAULTS["act_frac"],
    stats=DEFAULTS["stats"],
    reps=1,  # repeat the main loop (for slope-based benchmarking)
):
    """Trace the per-core CIN program.  Returns the Bass module."""
    Q = P // G  # partitions per channel
    F = HW_ // Q  # free elems per partition
    NT = C_ // G  # number of tiles
    assert P % G == 0 and HW_ % Q == 0 and C_ % G == 0

    # ACT applies [0:FA), DVE applies [FA:F)
    FA = int(F * act_frac) // 2 * 2
    assert 0 < FA < F

    o_g4, o_e4, o_gam, o_bet, o_sw, NCOLS = _const_layout(C_, S_, G)

    nc = bacc.Bacc(trn_type="TRN2")

    x_d = nc.dram_tensor("x", [C_ * Q, F], f16, kind="ExternalInput")
    consts_d = nc.dram_tensor("consts", [P, NCOLS], f32, kind="ExternalInput")
    y_d = nc.dram_tensor("y", [C_ * Q, F], f16, kind="ExternalOutput")

    with tile.TileContext(nc) as tc, ExitStack() as ctx:
        xpool = ctx.enter_context(tc.tile_pool(name="xt", bufs=xt_bufs))
        sqpool = ctx.enter_context(tc.tile_pool(name="sq", bufs=1))
        ppool = ctx.enter_context(tc.tile_pool(name="part", bufs=4))
        stpool = ctx.enter_context(tc.tile_pool(name="st", bufs=4))
        sbpool = ctx.enter_context(tc.tile_pool(name="sb", bufs=4))
        singles = ctx.enter_context(tc.tile_pool(name="singles", bufs=1))
        ch_ps = ctx.enter_context(tc.tile_pool(name="chps", bufs=2, space="PSUM"))
        bc_ps = ctx.enter_context(tc.tile_pool(name="bcps", bufs=2, space="PSUM"))
        gb_psp = ctx.enter_context(tc.tile_pool(name="gbps", bufs=1, space="PSUM"))

        # ---- constants: one DMA + one DVE funnel copy ----
        consts_sb = singles.tile([P, NCOLS], f32)
        nc.gpsimd.dma_start(out=consts_sb[:], in_=consts_d[:])
        consts_f = singles.tile([P, NCOLS], f32)
        nc.vector.tensor_copy(consts_f[:], consts_sb[:])

        g4_f = consts_f[:, o_g4 : o_g4 + G]  # [128, G] selector, 1/HW entries
        e4_f = consts_f[0:G, o_e4 : o_e4 + P]  # [G, 128] expander, 0/1 entries
        sw_f = consts_f[0:S_, o_sw : o_sw + 1]  # [S, 1]

        eps_sb = singles.tile([G, 1], f32)
        nc.vector.memset(eps_sb[:], EPS)

        # gb_all[:, t, 0] = gamma_mix for tile t's channels, [:, t, 1] = beta_mix
        gb_ps = gb_psp.tile([G, NT, 2], f32)
        gb_all = singles.tile([G, NT, 2], f32)
        for t in range(NT):
            gam_t = consts_f[0:S_, o_gam + G * t : o_gam + G * (t + 1)]
            bet_t = consts_f[0:S_, o_bet + G * t : o_bet + G * (t + 1)]
            nc.tensor.matmul(gb_ps[:, t, 0:1], gam_t, sw_f, start=True, stop=True)
            nc.tensor.matmul(gb_ps[:, t, 1:2], bet_t, sw_f, start=True, stop=True)
        nc.vector.tensor_copy(gb_all[:], gb_ps[:])

        # ---- main loop over channel tiles ----
        for t in [t for _ in range(reps) for t in range(NT)]:
            xt = xpool.tile([P, F], f16)
            nc.sync.dma_start(out=xt[:], in_=x_d[P * t : P * (t + 1), :])

            if stats == "copy":  # DMA-roofline probe: no compute at all
                nc.gpsimd.dma_start(out=y_d[P * t : P * (t + 1), :], in_=xt[:])
                continue

            # per-partition sum and sum-of-squares
            if stats == "split3":
                # sum split DVE/Pool, sumsq on ACT; fold all via one matmul
                FR = F // 2
                part = ppool.tile([P, 3], f32)
                nc.vector.reduce_sum(
                    part[:, 0:1], xt[:, 0:FR], axis=mybir.AxisListType.X
                )
                nc.gpsimd.reduce_sum(
                    part[:, 1:2], xt[:, FR:F], axis=mybir.AxisListType.X
                )
                sq = sqpool.tile([P, F], f16)
                nc.scalar.activation(
                    out=sq[:], in_=xt[:], func=AF.Square, accum_out=part[:, 2:3]
                )
                part2 = ppool.tile([P, 3], f32, tag="part2")
                nc.vector.tensor_copy(part2[:], part[:])

                ch = ch_ps.tile([G, 3], f32)
                nc.tensor.matmul(ch[:], g4_f, part2[:], start=True, stop=True)

                # st columns: 0=mean 1=exsq 2=tmp 3=var 4=scale 5=bias 6=std 7=rstd
                st = stpool.tile([G, 8], f32)
                stt = stpool.tile([G, 3], f32, tag="stt")
                nc.vector.tensor_copy(stt[:], ch[:])
                nc.vector.tensor_add(st[:, 0:1], stt[:, 0:1], stt[:, 1:2])
                nc.vector.tensor_copy(st[:, 1:2], stt[:, 2:3])
            else:
                part = ppool.tile([P, 2], f32)
                if stats == "act_square":
                    nc.vector.reduce_sum(
                        part[:, 0:1], xt[:], axis=mybir.AxisListType.X
                    )
                    sq = sqpool.tile([P, F], f16)
                    nc.scalar.activation(
                        out=sq[:], in_=xt[:], func=AF.Square, accum_out=part[:, 1:2]
                    )
                    # funnel both stats through DVE: PE matmul needs one wait
                    part2 = ppool.tile([P, 2], f32, tag="part2")
                    nc.vector.tensor_copy(part2[:], part[:])
                elif stats == "dve_ts":  # both stats on DVE
                    sq = sqpool.tile([P, F], f16)
                    nc.vector.tensor_scalar(
                        out=sq[:], in0=xt[:], scalar1=1.0, scalar2=None,
                        op0=mybir.AluOpType.mult, op1=mybir.AluOpType.add,
                        accum_out=part[:, 0:1],
                    )
                    nc.vector.tensor_tensor_reduce(
                        out=sq[:], in0=xt[:], in1=xt[:], scale=1.0, scalar=0.0,
                        op0=mybir.AluOpType.mult, op1=mybir.AluOpType.add,
                        accum_out=part[:, 1:2],
                    )
                    part2 = part  # already DVE-produced; matmul needs one wait
                else:  # "ts_square": sum on DVE ts-accum, sumsq on ACT
                    sq = sqpool.tile([P, F], f16)
                    nc.vector.tensor_scalar(
                        out=sq[:], in0=xt[:], scalar1=1.0, scalar2=None,
                        op0=mybir.AluOpType.mult, op1=mybir.AluOpType.add,
                        accum_out=part[:, 0:1],
                    )
                    sq2 = sqpool.tile([P, F], f16, tag="sq2")
                    nc.scalar.activation(
                        out=sq2[:], in_=xt[:], func=AF.Square, accum_out=part[:, 1:2]
                    )
                    # funnel both stats through DVE: PE matmul needs one wait
                    part2 = ppool.tile([P, 2], f32, tag="part2")
                    nc.vector.tensor_copy(part2[:], part[:])

                # fold Q partitions -> per-channel (mean, E[x^2])
                ch = ch_ps.tile([G, 2], f32)
                nc.tensor.matmul(ch[:], g4_f, part2[:], start=True, stop=True)

                # st cols: 0=mean 1=exsq 2=tmp 3=var 4=scale 5=bias 6=std 7=rstd
                st = stpool.tile([G, 8], f32)
                nc.vector.tensor_copy(st[:, 0:2], ch[:])

            nc.vector.tensor_mul(st[:, 2:3], st[:, 0:1], st[:, 0:1])
            nc.vector.tensor_sub(st[:, 3:4], st[:, 1:2], st[:, 2:3])
            nc.scalar.activation(
                out=st[:, 6:7], in_=st[:, 3:4], func=AF.Sqrt, bias=eps_sb[:]
            )
            nc.vector.reciprocal(st[:, 7:8], st[:, 6:7])
            nc.vector.tensor_mul(st[:, 4:5], st[:, 7:8], gb_all[:, t, 0:1])
            nc.vector.tensor_mul(st[:, 2:3], st[:, 0:1], st[:, 4:5])
            nc.vector.tensor_sub(st[:, 5:6], gb_all[:, t, 1:2], st[:, 2:3])

            # broadcast per-channel (scale, bias) back to the Q partitions each
            bc = bc_ps.tile([P, 2], f32)
            nc.tensor.matmul(bc[:], e4_f, st[:, 4:6], start=True, stop=True)
            sb2 = sbpool.tile([P, 2], f32)
            nc.vector.tensor_copy(sb2[:], bc[:])

            # y = scale * x + bias, in place.  ACT handles [0:FA), DVE
            # handles [FA:F) concurrently; each segment is stored as soon
            # as its engine finishes.
            nc.scalar.activation(
                out=xt[:, 0:FA], in_=xt[:, 0:FA], func=AF.Identity,
                bias=sb2[:, 1:2], scale=sb2[:, 0:1],
            )
            nc.gpsimd.dma_start(out=y_d[P * t : P * (t + 1), 0:FA], in_=xt[:, 0:FA])
            nc.vector.tensor_scalar(
                out=xt[:, FA:F], in0=xt[:, FA:F],
                scalar1=sb2[:, 0:1], scalar2=sb2[:, 1:2],
                op0=mybir.AluOpType.mult, op1=mybir.AluOpType.add,
            )
            nc.gpsimd.dma_start(out=y_d[P * t : P * (t + 1), FA:F], in_=xt[:, FA:F])

    nc.compile()
    return nc


def make_consts(C_=C, HW_=HW, S_=S, G=DEFAULTS["G"], gammas=None, betas=None, sw=None):
    """Host-side packed constants tensor [128, NCOLS]."""
    Q = P // G
    o_g4, o_e4, o_gam, o_bet, o_sw, NCOLS = _const_layout(C_, S_, G)
    consts = np.zeros((P, NCOLS), np.float32)
    consts[np.arange(P), o_g4 + np.arange(P) // Q] = 1.0 / HW_
    consts[np.arange(P) // Q, o_e4 + np.arange(P)] = 1.0
    consts[0:S_, o_gam : o_gam + C_] = gammas
    consts[0:S_, o_bet : o_bet + C_] = betas
    consts[0:S_, o_sw] = sw
    return consts


_CACHE = {}


def _get_nc():
    if "nc" not in _CACHE:
        _CACHE["nc"] = build_cin_program()
    return _CACHE["nc"]


def kernel(x, style_weights, gammas, betas, _trace=False):
    x = np.asarray(x, dtype=np.float32)
    style_weights = np.ascontiguousarray(np.asarray(style_weights, dtype=np.float32))
    gammas = np.ascontiguousarray(np.asarray(gammas, dtype=np.float32))
    betas = np.ascontiguousarray(np.asarray(betas, dtype=np.float32))

    G = DEFAULTS["G"]
    Q = P // G
    F = HW // Q
    nc = _get_nc()

    x16 = np.ascontiguousarray(x.astype(np.float16))
    xr = x16.reshape(B, C * Q, F)
    in_maps = [
        {
            "x": xr[i],
            "consts": make_consts(C, HW, S, G, gammas, betas, style_weights[i]),
        }
        for i in range(N_CORES)
    ]
    res = run_bass_kernel_spmd(
        nc, in_maps, core_ids=list(range(N_CORES)), trace=_trace
    )
    y = np.stack(
        [
            res.results[i]["y"].astype(np.float32).reshape(C, H, W)
            for i in range(N_CORES)
        ],
        axis=0,
    )
    if _trace:
        return y, res
    return y


# revision 14
# speedup vs baseline: 1.5921x; 1.5921x over previous
"""Conditional Instance Norm (CIN) kernel for Trainium2, data-parallel over batch.

Reference semantics (per batch sample b, channel c):
    gamma_mix = style_weights @ gammas          # [B, C]
    beta_mix  = style_weights @ betas           # [B, C]
    y[b,c]    = gamma_mix[b,c] * (x[b,c] - mean) * rsqrt(var + eps) + beta_mix[b,c]
with mean/var over the spatial dims of x[b,c] (biased var).

Strategy: one batch sample per NeuronCore (B=8 samples, 8 cores).  HBM I/O is
fp16 (host converts): rel-err budget is 2e-2 and fp16 quantization costs
~1e-3, so this halves the memory-roofline floor vs fp32 — 32 MiB read +
32 MiB write per core instead of 64+64.  Channels are processed in tiles of
G channels; each channel's HW elements are laid out over Q=128/G partitions,
so a tile is a dense [128, F=HW/Q] fp16 SBUF block read from HBM exactly once
and written exactly once.

Per tile (default stats="act_square", act_frac=0.5, xt_bufs=10):
  DVE reduce_sum               -> per-partition sums   [128,1] f32
  ACT Square w/ accum_out      -> per-partition sumsq  [128,1] f32 (square
                                  result dumped to a fp16 scratch, never read)
  PE matmul w/ 1/HW selector   -> per-channel (mean, E[x^2])  [G,2]
  tiny DVE/ACT ops             -> scale = gamma*rsqrt(var+eps),
                                  bias  = beta - mean*scale   [G,2]
  PE matmul w/ 0/1 expander    -> per-partition (scale, bias) [128,2]
  apply y = scale*x + bias, split 50/50 between ACT (Identity w/ scale+bias
  APs) and DVE (tensor_scalar mult+add, fast fp16 mode) so neither engine
  exceeds the per-tile HBM budget; each segment is stored as soon as done.

PE Matmult instructions only tolerate a single sync-wait, so every matmul
operand is funneled through a DVE-produced tile (one wait, one engine).

HW notes (measured on trn2, slope bench):
  f32 baseline 404us; fp16 DMA-only copy ~210-216us (fp16 DMA runs at
  bytes-rate); this kernel ~228us.  tensor_tensor_reduce CRASHES on HW
  (works in CoreSim) — do not use.  tensor_scalar with accum_out works on
  HW but its mandatory full-size output write makes it a net loss vs
  reduce_sum (ts_square/dve_ts schemes measured 277us).  Pool (gpsimd)
  tensor_reduce only supports partition-axis (C) reductions.  TimelineSim
  under-predicts HW: f32 +8%, fp16 engine-heavy configs +20-50%.
"""

import sys

for _p in ("/opt/trn_rl_repo",):
    if _p not in sys.path:
        sys.path.insert(0, _p)

from contextlib import ExitStack

import numpy as np

import concourse.bacc as bacc
import concourse.tile as tile
from concourse import mybir
from concourse.bass_utils import run_bass_kernel_spmd

EPS = 1e-5

# Full problem dims (hardcoded per harness contract).
B, C, H, W = 8, 256, 256, 256
S = 16
HW = H * W
N_CORES = 8
P = 128  # SBUF partitions

AF = mybir.ActivationFunctionType
f32 = mybir.dt.float32
f16 = mybir.dt.float16


def _const_layout(C_, S_, G):
    """Column offsets of the packed constants tensor: g4 | e4 | gammas | betas | sw."""
    o_g4 = 0
    o_e4 = o_g4 + G
    o_gam = o_e4 + P
    o_bet = o_gam + C_
    o_sw = o_bet + C_
    ncols = o_sw + 1
    return o_g4, o_e4, o_gam, o_bet, o_sw, ncols


# act_frac: fraction of the apply pass done on the Scalar engine (rest on DVE).
# stats: "act_square" = DVE reduce_sum + ACT Square/accum (two full passes);
#        "dve_ts"     = DVE tensor_scalar*1.0/accum (sum) + DVE ttr x*x/accum
#                       (sumsq) — both candidates for the 2x/4x fp16 DVE modes,
#                       leaving ACT only the apply.
DEFAULTS = dict(G=16, xt_bufs=10, act_frac=0.35, stats="act_square")


def build_cin_program(
    C_=C,
    HW_=HW,
    S_=S,
    G=DEFAULTS["G"],  # channels per tile
    xt_bufs=DEFAULTS["xt_bufs"],
    act_frac=DEFAULTS["act_frac"],
    stats=DEFAULTS["stats"],
    reps=1,  # repeat the main loop (for slope-based benchmarking)
):
    """Trace the per-core CIN program.  Returns the Bass module."""
    Q = P // G  # partitions per channel
    F = HW_ // Q  # free elems per partition
    NT = C_ // G  # number of tiles
    assert P % G == 0 and HW_ % Q == 0 and C_ % G == 0

    # ACT applies [0:FA), DVE applies [FA:F)
    FA = int(F * act_frac) // 2 * 2
    assert 0 < FA < F

    o_g4, o_e4, o_gam, o_bet, o_sw, NCOLS = _const_layout(C_, S_, G)

    nc = bacc.Bacc(trn_type="TRN2")

    x_d = nc.dram_tensor("x", [C_ * Q, F], f16, kind="ExternalInput")
    consts_d = nc.dram_tensor("consts", [P, NCOLS], f32, kind="ExternalInput")
    y_d = nc.dram_tensor("y", [C_ * Q, F], f16, kind="ExternalOutput")

    with tile.TileContext(nc) as tc, ExitStack() as ctx:
        xpool = ctx.enter_context(tc.tile_pool(name="xt", bufs=xt_bufs))
        sqpool = ctx.enter_context(tc.tile_pool(name="sq", bufs=1))
        ppool = ctx.enter_context(tc.tile_pool(name="part", bufs=4))
        stpool = ctx.enter_context(tc.tile_pool(name="st", bufs=4))
        sbpool = ctx.enter_context(tc.tile_pool(name="sb", bufs=4))
        singles = ctx.enter_context(tc.tile_pool(name="singles", bufs=1))
        ch_ps = ctx.enter_context(tc.tile_pool(name="chps", bufs=2, space="PSUM"))
        bc_ps = ctx.enter_context(tc.tile_pool(name="bcps", bufs=2, space="PSUM"))
        gb_psp = ctx.enter_context(tc.tile_pool(name="gbps", bufs=1, space="PSUM"))

        # ---- constants: one DMA + one DVE funnel copy ----
        consts_sb = singles.tile([P, NCOLS], f32)
        nc.gpsimd.dma_start(out=consts_sb[:], in_=consts_d[:])
        consts_f = singles.tile([P, NCOLS], f32)
        nc.vector.tensor_copy(consts_f[:], consts_sb[:])

        g4_f = consts_f[:, o_g4 : o_g4 + G]  # [128, G] selector, 1/HW entries
        e4_f = consts_f[0:G, o_e4 : o_e4 + P]  # [G, 128] expander, 0/1 entries
        sw_f = consts_f[0:S_, o_sw : o_sw + 1]  # [S, 1]

        eps_sb = singles.tile([G, 1], f32)
        nc.vector.memset(eps_sb[:], EPS)

        # gb_all[:, t, 0] = gamma_mix for tile t's channels, [:, t, 1] = beta_mix
        gb_ps = gb_psp.tile([G, NT, 2], f32)
        gb_all = singles.tile([G, NT, 2], f32)
        for t in range(NT):
            gam_t = consts_f[0:S_, o_gam + G * t : o_gam + G * (t + 1)]
            bet_t = consts_f[0:S_, o_bet + G * t : o_bet + G * (t + 1)]
            nc.tensor.matmul(gb_ps[:, t, 0:1], gam_t, sw_f, start=True, stop=True)
            nc.tensor.matmul(gb_ps[:, t, 1:2], bet_t, sw_f, start=True, stop=True)
        nc.vector.tensor_copy(gb_all[:], gb_ps[:])

        # ---- main loop over channel tiles ----
        for t in [t for _ in range(reps) for t in range(NT)]:
            xt = xpool.tile([P, F], f16)
            nc.sync.dma_start(out=xt[:], in_=x_d[P * t : P * (t + 1), :])

            if stats == "copy":  # DMA-roofline probe: no compute at all
                nc.gpsimd.dma_start(out=y_d[P * t : P * (t + 1), :], in_=xt[:])
                continue

            # per-partition sum and sum-of-squares
            if stats == "split3":
                # sum split DVE/Pool, sumsq on ACT; fold all via one matmul
                FR = F // 2
                part = ppool.tile([P, 3], f32)
                nc.vector.reduce_sum(
                    part[:, 0:1], xt[:, 0:FR], axis=mybir.AxisListType.X
                )
                nc.gpsimd.reduce_sum(
                    part[:, 1:2], xt[:, FR:F], axis=mybir.AxisListType.X
                )
                sq = sqpool.tile([P, F], f16)
                nc.scalar.activation(
                    out=sq[:], in_=xt[:], func=AF.Square, accum_out=part[:, 2:3]
                )
                part2 = ppool.tile([P, 3], f32, tag="part2")
                nc.vector.tensor_copy(part2[:], part[:])

                ch = ch_ps.tile([G, 3], f32)
                nc.tensor.matmul(ch[:], g4_f, part2[:], start=True, stop=True)

                # st columns: 0=mean 1=exsq 2=tmp 3=var 4=scale 5=bias 6=std 7=rstd
                st = stpool.tile([G, 8], f32)
                stt = stpool.tile([G, 3], f32, tag="stt")
                nc.vector.tensor_copy(stt[:], ch[:])
                nc.vector.tensor_add(st[:, 0:1], stt[:, 0:1], stt[:, 1:2])
                nc.vector.tensor_copy(st[:, 1:2], stt[:, 2:3])
            else:
                part = ppool.tile([P, 2], f32)
                if stats == "act_square":
                    nc.vector.reduce_sum(
                        part[:, 0:1], xt[:], axis=mybir.AxisListType.X
                    )
                    sq = sqpool.tile([P, F], f16)
                    nc.scalar.activation(
                        out=sq[:], in_=xt[:], func=AF.Square, accum_out=part[:, 1:2]
                    )
                    # funnel both stats through DVE: PE matmul needs one wait
                    part2 = ppool.tile([P, 2], f32, tag="part2")
                    nc.vector.tensor_copy(part2[:], part[:])
                elif stats == "dve_ts":  # both stats on DVE
                    sq = sqpool.tile([P, F], f16)
                    nc.vector.tensor_scalar(
                        out=sq[:], in0=xt[:], scalar1=1.0, scalar2=None,
                        op0=mybir.AluOpType.mult, op1=mybir.AluOpType.add,
                        accum_out=part[:, 0:1],
                    )
                    nc.vector.tensor_tensor_reduce(
                        out=sq[:], in0=xt[:], in1=xt[:], scale=1.0, scalar=0.0,
                        op0=mybir.AluOpType.mult, op1=mybir.AluOpType.add,
                        accum_out=part[:, 1:2],
                    )
                    part2 = part  # already DVE-produced; matmul needs one wait
                else:  # "ts_square": sum on DVE ts-accum, sumsq on ACT
                    sq = sqpool.tile([P, F], f16)
                    nc.vector.tensor_scalar(
                        out=sq[:], in0=xt[:], scalar1=1.0, scalar2=None,
                        op0=mybir.AluOpType.mult, op1=mybir.AluOpType.add,
                        accum_out=part[:, 0:1],
                    )
                    sq2 = sqpool.tile([P, F], f16, tag="sq2")
                    nc.scalar.activation(
                        out=sq2[:], in_=xt[:], func=AF.Square, accum_out=part[:, 1:2]
                    )
                    # funnel both stats through DVE: PE matmul needs one wait
                    part2 = ppool.tile([P, 2], f32, tag="part2")
                    nc.vector.tensor_copy(part2[:], part[:])

                # fold Q partitions -> per-channel (mean, E[x^2])
                ch = ch_ps.tile([G, 2], f32)
                nc.tensor.matmul(ch[:], g4_f, part2[:], start=True, stop=True)

                # st cols: 0=mean 1=exsq 2=tmp 3=var 4=scale 5=bias 6=std 7=rstd
                st = stpool.tile([G, 8], f32)
                nc.vector.tensor_copy(st[:, 0:2], ch[:])

            nc.vector.tensor_mul(st[:, 2:3], st[:, 0:1], st[:, 0:1])
            nc.vector.tensor_sub(st[:, 3:4], st[:, 1:2], st[:, 2:3])
            nc.scalar.activation(
                out=st[:, 6:7], in_=st[:, 3:4], func=AF.Sqrt, bias=eps_sb[:]
            )
            nc.vector.reciprocal(st[:, 7:8], st[:, 6:7])
            nc.vector.tensor_mul(st[:, 4:5], st[:, 7:8], gb_all[:, t, 0:1])
            nc.vector.tensor_mul(st[:, 2:3], st[:, 0:1], st[:, 4:5])
            nc.vector.tensor_sub(st[:, 5:6], gb_all[:, t, 1:2], st[:, 2:3])

            # broadcast per-channel (scale, bias) back to the Q partitions each
            bc = bc_ps.tile([P, 2], f32)
            nc.tensor.matmul(bc[:], e4_f, st[:, 4:6], start=True, stop=True)
            sb2 = sbpool.tile([P, 2], f32)
            nc.vector.tensor_copy(sb2[:], bc[:])

            # y = scale * x + bias, in place.  ACT handles [0:FA), DVE
            # handles [FA:F) concurrently; each segment is stored as soon
            # as its engine finishes.
            nc.scalar.activation(
                out=xt[:, 0:FA], in_=xt[:, 0:FA], func=AF.Identity,
                bias=sb2[:, 1:2], scale=sb2[:, 0:1],
            )
            nc.gpsimd.dma_start(out=y_d[P * t : P * (t + 1), 0:FA], in_=xt[:, 0:FA])
            nc.vector.tensor_scalar(
                out=xt[:, FA:F], in0=xt[:, FA:F],
                scalar1=sb2[:, 0:1], scalar2=sb2[:, 1:2],
                op0=mybir.AluOpType.mult, op1=mybir.AluOpType.add,
            )
            nc.gpsimd.dma_start(out=y_d[P * t : P * (t + 1), FA:F], in_=xt[:, FA:F])

    nc.compile()
    return nc


def make_consts(C_=C, HW_=HW, S_=S, G=DEFAULTS["G"], gammas=None, betas=None, sw=None):
    """Host-side packed constants tensor [128, NCOLS]."""
    Q = P // G
    o_g4, o_e4, o_gam, o_bet, o_sw, NCOLS = _const_layout(C_, S_, G)
    consts = np.zeros((P, NCOLS), np.float32)
    consts[np.arange(P), o_g4 + np.arange(P) // Q] = 1.0 / HW_
    consts[np.arange(P) // Q, o_e4 + np.arange(P)] = 1.0
    consts[0:S_, o_gam : o_gam + C_] = gammas
    consts[0:S_, o_bet : o_bet + C_] = betas
    consts[0:S_, o_sw] = sw
    return consts


_CACHE = {}


def _get_nc():
    if "nc" not in _CACHE:
        _CACHE["nc"] = build_cin_program()
    return _CACHE["nc"]


def kernel(x, style_weights, gammas, betas, _trace=False):
    x = np.asarray(x, dtype=np.float32)
    style_weights = np.ascontiguousarray(np.asarray(style_weights, dtype=np.float32))
    gammas = np.ascontiguousarray(np.asarray(gammas, dtype=np.float32))
    betas = np.ascontiguousarray(np.asarray(betas, dtype=np.float32))

    G = DEFAULTS["G"]
    Q = P // G
    F = HW // Q
    nc = _get_nc()

    x16 = np.ascontiguousarray(x.astype(np.float16))
    xr = x16.reshape(B, C * Q, F)
    in_maps = [
        {
            "x": xr[i],
            "consts": make_consts(C, HW, S, G, gammas, betas, style_weights[i]),
        }
        for i in range(N_CORES)
    ]
    res = run_bass_kernel_spmd(
        nc, in_maps, core_ids=list(range(N_CORES)), trace=_trace
    )
    y = np.stack(
        [
            res.results[i]["y"].astype(np.float32).reshape(C, H, W)
            for i in range(N_CORES)
        ],
        axis=0,
    )
    if _trace:
        return y, res
    return y
